# revision 6
# baseline (speedup 1.0000x reference)
"""Binarized 3x3 conv (BinaryConnect) on 8 Trainium2 NeuronCores.

Problem: y = conv2d(x, sign(w), stride=1, pad=1) + bias
  x: (32, 256, 56, 56) f32, w: (256, 256, 3, 3) f32, bias: (256,) f32
  out: (32, 256, 56, 56) f32

Strategy (data-parallel over batch, 4 images/core), F(4,3) Winograd
along H + direct along W, all-fp16 datapath (same PE rate as bf16,
8x less quantization error, which the Winograd cancellations need):

  - Host: binarize weights, transform along r with G (F(4,3)),
    cast x to fp16, zero-pad each 56x56 plane into the pitch-57
    layout (shared pad column) used by the baseline.
  - Device, per image/channel-group: DVE+GpSimd compute the 6-point
    B^T forward transform along H into U planes [14 hb x 57] (one
    plane per Winograd point u; W-taps stay direct so every matmul
    rhs is a contiguous shift of a U plane). Per (kg, u, chunk):
    6 matmuls (3 s-taps x 2 cg) of 399 cols accumulate M_u in PSUM;
    the 6 u-points are processed in halves of 3 so at most 6+2 PSUM
    banks are live. Inverse A^T transform runs on DVE/GpSimd
    (scalar_tensor_tensor for the x2/x4/x8 terms), scalar engine
    applies bias + crops the pitch column + interleaves the 4
    Winograd output rows into packed 56x56 planes, DMA out.
  - PE work halves vs direct conv: 6 pts x 3 taps vs 9 taps x
    K=128-pairs => 229,824 vs 459,648 matmul columns per core.
"""

import numpy as np

import concourse.bacc as bacc
import concourse.mybir as mybir
from concourse.tile import TileContext
from concourse.bass_utils import run_bass_kernel_spmd

# problem constants (hardcoded per harness contract)
N_IMG = 32
C = 256  # input channels
K = 256  # output channels
H = W = 56
HP = 58  # padded rows (1 top + 56 + 1 bottom)
WP = 57  # row pitch: 1 shared pad column + 56 data
N_CORES = 8
IMG_PER_CORE = N_IMG // N_CORES

L_PLANE = HP * WP  # 3306
L_PAD = L_PLANE + 4  # 3310
LEAD = 2  # leading slack so tap offset (-1) stays in-bounds
X_SLACK = 188  # tail slack so the strided d_i views can be constructed
HB = 14  # h-blocks of 4 output rows
CHUNK = 7 * WP  # 399 cols per PSUM chunk (2 chunks per plane)
UPL = HB * WP  # 798: one U plane
U_LEAD = 2
UL = U_LEAD + UPL + 4  # 804 pitch between U planes
VL = 800  # pitch between v planes in the vstage tile
NW = 36  # distinct (u, s, cg) weight tiles

FP16 = mybir.dt.float16
F32 = mybir.dt.float32

# F(4,3) weight transform (Lavin / wincnn convention)
G_MAT = np.array(
    [
        [1 / 4, 0, 0],
        [-1 / 6, -1 / 6, -1 / 6],
        [-1 / 6, 1 / 6, -1 / 6],
        [1 / 24, 1 / 12, 1 / 6],
        [1 / 24, -1 / 12, 1 / 6],
        [0, 0, 1],
    ],
    np.float64,
)

_compiled = {}


def _ldw_key(inst):
    ap = inst.ins[0]
    bap = getattr(ap, "bass_ap", None)
    if bap is not None:
        try:
            return (bap.tensor.name, bap.offset, str(bap.ap), str(ap.dtype))
        except AttributeError:
            return None
    try:
        return (ap.memref, ap.offset, str(ap.ap), str(ap.dtype))
    except AttributeError:
        return None


def _dedup_ldweights(ordered):
    """Drop InstLdweights that reload weights already resident in the PE
    array. Only drops clean instructions: no sync_info and sync-deps
    covered by the retained load."""
    n_drop = 0
    for bb, insts in ordered.items():
        out = []
        last_key = None
        last_deps = None
        for inst in insts:
            if isinstance(inst, mybir.InstLdweights):
                key = _ldw_key(inst)
                si = inst.sync_info
                clean = si is None or (not si.on_wait and not si.on_update)
                sdeps = set(inst.sync_dependency_names())
                nsdeps = set(inst.nosync_dependency_names())
                if (
                    key is not None
                    and key == last_key
                    and clean
                    and last_deps is not None
                    and sdeps <= last_deps
                    and not nsdeps
                ):
                    n_drop += 1
                    continue
                last_key = key
                last_deps = sdeps
            elif isinstance(inst, mybir.InstMatmult):
                pass  # does not clobber the weight array
            elif getattr(inst, "engine", None) == mybir.EngineType.PE:
                last_key = None
            out.append(inst)
        ordered[bb] = out
    return n_drop


def _build_bass(loops=1):
    import concourse.tile as tile_mod

    nc = bacc.Bacc()

    xp = nc.declare_dram_parameter("xp", [IMG_PER_CORE, 2, 128, L_PAD], FP16, isOutput=False)
    wt = nc.declare_dram_parameter("wt", [128, NW * 256], FP16, isOutput=False)
    bi = nc.declare_dram_parameter("bi", [2, 128, 1], F32, isOutput=False)
    y = nc.declare_dram_parameter("y", [IMG_PER_CORE, 2, 128, H * W], F32, isOutput=True)

    orig_legalize = tile_mod.tile_legalize

    def legalize_and_dedup(ordered, nc_arg):
        ordered = orig_legalize(ordered, nc_arg)
        _dedup_ldweights(ordered)
        return ordered

    tile_mod.tile_legalize = legalize_and_dedup
    try:
        _build_tile_program(nc, loops, xp, wt, bi, y)
    finally:
        tile_mod.tile_legalize = orig_legalize
    nc.compile()
    return nc


def _dview(xs, i):
    """x rows 4*hb + i (padded index) as a [128, 14, 57] strided view."""
    sl = xs[:, LEAD + i * WP : LEAD + i * WP + HB * 4 * WP]
    return sl.rearrange("p (hb f w) -> p hb f w", f=4, w=WP)[:, :, 0, :]


def _uview(us, u):
    return us[:, u * UL + U_LEAD : u * UL + U_LEAD + UPL].rearrange(
        "p (hb w) -> p hb w", w=WP
    )


def _fwd_transform(nc, xs, us, ts):
    """U_u = B^T d along H. DVE: u0,u3,u4,u5 (8 ops); GpSimd: u1,u2 (6 ops)."""
    A = mybir.AluOpType
    d = [_dview(xs, i) for i in range(6)]
    U = [_uview(us, u) for u in range(6)]
    T0 = ts[:, 0:UPL].rearrange("p (hb w) -> p hb w", w=WP)
    T1 = ts[:, VL : VL + UPL].rearrange("p (hb w) -> p hb w", w=WP)
    v, g = nc.vector, nc.gpsimd
    # u0 = -5*d2 + (4*d0 + d4)
    v.scalar_tensor_tensor(T0, d[0], 4.0, d[4], A.mult, A.add)
    v.scalar_tensor_tensor(U[0], d[2], -5.0, T0, A.mult, A.add)
    # u1 = -4*(d1+d2) + (d3+d4)
    g.tensor_add(U[1], d[1], d[2])
    g.tensor_add(T1, d[3], d[4])
    v.scalar_tensor_tensor(U[1], U[1], -4.0, T1, A.mult, A.add)
    # u2 = 4*(d1-d2) + (d4-d3)
    g.tensor_sub(U[2], d[1], d[2])
    g.tensor_sub(T1, d[4], d[3])
    v.scalar_tensor_tensor(U[2], U[2], 4.0, T1, A.mult, A.add)
    # u3 = 2*(d3-d1) + (d4-d2);  u4 = -2*(d3-d1) + (d4-d2)
    v.tensor_sub(U[3], d[3], d[1])
    v.tensor_sub(T0, d[4], d[2])
    v.scalar_tensor_tensor(U[4], U[3], -2.0, T0, A.mult, A.add)
    v.scalar_tensor_tensor(U[3], U[3], 2.0, T0, A.mult, A.add)
    # u5 = -5*d3 + (4*d1 + d5)
    v.scalar_tensor_tensor(T0, d[1], 4.0, d[5], A.mult, A.add)
    v.scalar_tensor_tensor(U[5], d[3], -5.0, T0, A.mult, A.add)


def _conv_kg(nc, pspool, it, vt, ot, usb0, usb1, wtile, bsb, y, n, kg):
    """GEMMs + inverse transform + bias/pack/DMA for one (img, kg)."""
    A = mybir.AluOpType
    usb = (usb0, usb1)

    def iv(j, ch):  # intermediate planes: a, b, m0, c, d
        return it[:, (j * 2 + ch) * 400 : (j * 2 + ch) * 400 + CHUNK]

    def vv(vi, ch):
        return vt[:, vi * VL + ch * CHUNK : vi * VL + (ch + 1) * CHUNK]

    for half in range(2):
        upts = (0, 1, 2) if half == 0 else (3, 4, 5)
        ps = {}
        for u in upts:
            for ch in range(2):
                ps[u, ch] = pspool.tile(
                    [128, CHUNK], F32, tag="ps", name=f"ps_{n}_{kg}_{u}_{ch}"
                )
        for u in upts:
            for si, (s, cgi) in enumerate(
                (s, c) for s in range(3) for c in range(2)
            ):
                wsl = wtile[
                    :,
                    ((u * 3 + s) * 2 + cgi) * 256
                    + kg * 128 : ((u * 3 + s) * 2 + cgi) * 256
                    + kg * 128
                    + 128,
                ]
                for ch in range(2):
                    base = u * UL + U_LEAD + ch * CHUNK + s - 1
                    nc.tensor.matmul(
                        ps[u, ch][:],
                        lhsT=wsl,
                        rhs=usb[cgi][:, base : base + CHUNK],
                        start=(si == 0),
                        stop=(si == 5),
                    )
        # NCC_IBVF027: tensor-tensor ops may read only one input from PSUM,
        # so stage M1/M3 into SBUF on the scalar engine first.
        if half == 0:
            for ch in range(2):
                m1s = iv(5, ch)
                nc.scalar.copy(m1s, ps[1, ch][:])
                nc.vector.tensor_add(iv(0, ch), m1s, ps[2, ch][:])
                nc.vector.tensor_sub(iv(1, ch), m1s, ps[2, ch][:])
                nc.scalar.copy(iv(2, ch), ps[0, ch][:])
        else:
            for ch in range(2):
                a_, b_, m0 = iv(0, ch), iv(1, ch), iv(2, ch)
                c_, d_ = iv(3, ch), iv(4, ch)
                m3s = iv(6, ch)
                nc.scalar.copy(m3s, ps[3, ch][:])
                nc.vector.tensor_add(c_, m3s, ps[4, ch][:])
                nc.vector.tensor_sub(d_, m3s, ps[4, ch][:])
                nc.gpsimd.tensor_add(vv(0, ch), m0, a_)
                nc.gpsimd.tensor_add(vv(0, ch), vv(0, ch), c_)
                nc.vector.scalar_tensor_tensor(vv(1, ch), d_, 2.0, b_, A.mult, A.add)
                nc.vector.scalar_tensor_tensor(vv(2, ch), c_, 4.0, a_, A.mult, A.add)
                nc.vector.scalar_tensor_tensor(vv(3, ch), d_, 8.0, b_, A.mult, A.add)
                nc.vector.tensor_add(vv(3, ch), vv(3, ch), ps[5, ch][:])
    for vi in range(4):
        in_v = vt[:, vi * VL : vi * VL + UPL].rearrange("p (hb w) -> p hb w", w=WP)[
            :, :, 1 : 1 + W
        ]
        out_v = ot.rearrange("p (hb f w) -> p hb f w", f=4, w=W)[:, :, vi, :]
        nc.scalar.activation(
            out_v, in_v, mybir.ActivationFunctionType.Identity, bias=bsb[kg]
        )
    nc.sync.dma_start(out=y[n, kg], in_=ot[:])


def _build_tile_program(nc, loops, xp, wt, bi, y):
    with TileContext(nc) as tc:
        with (
            tc.tile_pool(name="wpool", bufs=1) as wpool,
            tc.tile_pool(name="xpool", bufs=1) as xpool,
            tc.tile_pool(name="upool", bufs=1) as upool,
            tc.tile_pool(name="tpool", bufs=1) as tpool,
            tc.tile_pool(name="ipool", bufs=2) as ipool,
            tc.tile_pool(name="vpool", bufs=2) as vpool,
            tc.tile_pool(name="opool", bufs=3) as opool,
            tc.tile_pool(name="pspool", bufs=8, space="PSUM") as pspool,
        ):
            wtile = wpool.tile([128, NW * 256], FP16, tag="w")
            nc.sync.dma_start(out=wtile[:], in_=wt[:])
            bsb = []
            for cg in range(2):
                btile = wpool.tile([128, 1], F32, tag=f"b{cg}")
                nc.sync.dma_start(out=btile[:], in_=bi[cg])
                bsb.append(btile)

            xsb, usb, tsb = {}, {}, {}
            for slot in range(2):
                for cg in range(2):
                    xsb[slot, cg] = xpool.tile(
                        [128, LEAD + L_PAD + X_SLACK], FP16, tag=f"x{slot}{cg}", name=f"xsb{slot}{cg}"
                    )
                    nc.vector.memset(xsb[slot, cg][:], 0.0)
                    usb[slot, cg] = upool.tile([128, 6 * UL], FP16, tag=f"u{slot}{cg}", name=f"usb{slot}{cg}")
                    nc.vector.memset(usb[slot, cg][:], 0.0)
                    tsb[slot, cg] = tpool.tile([128, 2 * VL], FP16, tag=f"t{slot}{cg}", name=f"tsb{slot}{cg}")

            import contextlib

            loop_cm = (
                tc.For_i(0, loops, 1, hint_engines=(mybir.EngineType.PE,))
                if loops > 1
                else contextlib.nullcontext()
            )
            with loop_cm:
                for n in range(IMG_PER_CORE):
                    slot = n % 2
                    for cg in range(2):
                        nc.sync.dma_start(
                            out=xsb[slot, cg][:, LEAD : LEAD + L_PAD], in_=xp[n, cg]
                        )
                    for cg in range(2):
                        _fwd_transform(nc, xsb[slot, cg], usb[slot, cg], tsb[slot, cg])
                    for kg in range(2):
                        it = ipool.tile([128, 14 * 400], FP16, tag="i")
                        vt = vpool.tile([128, 4 * VL], FP16, tag="v")
                        ot = opool.tile([128, H * W], F32, tag="o")
                        _conv_kg(
                            nc, pspool, it, vt, ot,
                            usb[slot, 0], usb[slot, 1],
                            wtile, bsb, y, n, kg,
                        )


def _get_compiled(loops=1):
    key = (loops,)
    if key not in _compiled:
        _compiled[key] = _build_bass(loops)
    return _compiled[key]


def _prepare_inputs(x, weight, bias):
    x = np.asarray(x, dtype=np.float32)
    weight = np.asarray(weight, dtype=np.float32)
    bias = np.asarray(bias, dtype=np.float32)

    # padded pitch-57 fp16 activations
    xp = np.zeros((N_IMG, C, L_PAD), dtype=np.float16)
    xp_img = xp[:, :, :L_PLANE].reshape(N_IMG, C, HP, WP)
    xp_img[:, :, 1 : 1 + H, 1 : 1 + W] = x.astype(np.float16)

    # Winograd-transformed binarized weights:
    # wt[c', ((u*3+s)*2+cg)*256 + k] = sum_r G[u,r] sign(w)[k, cg*128+c', r, s]
    g = np.sign(weight)  # [K, C, 3, 3]
    Wt = np.einsum("ur,kcrs->ucsk", G_MAT, g.astype(np.float64))  # [6, C, 3, K]
    arr = Wt.reshape(6, 2, 128, 3, K).transpose(2, 0, 3, 1, 4)  # [128, 6, 3, 2, K]
    wt = np.ascontiguousarray(arr.reshape(128, NW * 256)).astype(np.float16)

    bi = bias.astype(np.float32).reshape(2, 128, 1)
    return xp, wt, bi


def kernel(x, weight, bias, _trace=False, _trace_kwargs=None):
    nc = _get_compiled()
    xp, wt, bi = _prepare_inputs(x, weight, bias)

    in_maps = []
    for i in range(N_CORES):
        xs = np.ascontiguousarray(
            xp[i * IMG_PER_CORE : (i + 1) * IMG_PER_CORE].reshape(
                IMG_PER_CORE, 2, 128, L_PAD
            )
        )
        in_maps.append({"xp": xs, "wt": wt, "bi": bi})

    res = run_bass_kernel_spmd(
        nc, in_maps, list(range(N_CORES)), trace=_trace, **(_trace_kwargs or {})
    )
    out = np.concatenate(
        [r["y"].reshape(IMG_PER_CORE, K, H, W) for r in res.results], axis=0
    )
    if _trace:
        return np.asarray(out, dtype=np.float32), res
    return np.asarray(out, dtype=np.float32)


# revision 8
# speedup vs baseline: 1.1706x; 1.1706x over previous
"""Binarized 3x3 conv (BinaryConnect) on 8 Trainium2 NeuronCores.

Problem: y = conv2d(x, sign(w), stride=1, pad=1) + bias
  x: (32, 256, 56, 56) f32, w: (256, 256, 3, 3) f32, bias: (256,) f32
  out: (32, 256, 56, 56) f32

Strategy (data-parallel over batch, 4 images/core), F(4,3) Winograd
along H + direct along W, all-fp16 datapath (same PE rate as bf16,
8x less quantization error, which the Winograd cancellations need):

  - Host: binarize weights, transform along r with G (F(4,3)),
    cast x to fp16, zero-pad each 56x56 plane into the pitch-57
    layout (shared pad column) used by the baseline.
  - Device, per image/channel-group: DVE+GpSimd compute the 6-point
    B^T forward transform along H into U planes [14 hb x 57] (one
    plane per Winograd point u; W-taps stay direct so every matmul
    rhs is a contiguous shift of a U plane). Per (kg, u, chunk):
    6 matmuls (3 s-taps x 2 cg) of 399 cols accumulate M_u in PSUM;
    the 6 u-points are processed in halves of 3 so at most 6+2 PSUM
    banks are live. Inverse A^T transform runs on DVE/GpSimd
    (scalar_tensor_tensor for the x2/x4/x8 terms), scalar engine
    applies bias + crops the pitch column + interleaves the 4
    Winograd output rows into packed 56x56 planes, DMA out.
  - PE work halves vs direct conv: 6 pts x 3 taps vs 9 taps x
    K=128-pairs => 229,824 vs 459,648 matmul columns per core.
"""

import numpy as np

import concourse.bacc as bacc
import concourse.mybir as mybir
from concourse.tile import TileContext
from concourse.bass_utils import run_bass_kernel_spmd

# problem constants (hardcoded per harness contract)
N_IMG = 32
C = 256  # input channels
K = 256  # output channels
H = W = 56
HP = 58  # padded rows (1 top + 56 + 1 bottom)
WP = 57  # row pitch: 1 shared pad column + 56 data
N_CORES = 8
IMG_PER_CORE = N_IMG // N_CORES

L_PLANE = HP * WP  # 3306
L_PAD = L_PLANE + 4  # 3310
LEAD = 2  # leading slack so tap offset (-1) stays in-bounds
X_SLACK = 188  # tail slack so the strided d_i views can be constructed
HB = 14  # h-blocks of 4 output rows
CHUNK = 7 * WP  # 399 cols per PSUM chunk (2 chunks per plane)
UPL = HB * WP  # 798: one U plane
U_LEAD = 2
UL = U_LEAD + UPL + 4  # 804 pitch between U planes
VL = 800  # pitch between v planes in the vstage tile
NW = 36  # distinct (u, s, cg) weight tiles

FP16 = mybir.dt.float16
F32 = mybir.dt.float32

# F(4,3) weight transform (Lavin / wincnn convention)
G_MAT = np.array(
    [
        [1 / 4, 0, 0],
        [-1 / 6, -1 / 6, -1 / 6],
        [-1 / 6, 1 / 6, -1 / 6],
        [1 / 24, 1 / 12, 1 / 6],
        [1 / 24, -1 / 12, 1 / 6],
        [0, 0, 1],
    ],
    np.float64,
)

_compiled = {}


def _ldw_key(inst):
    ap = inst.ins[0]
    bap = getattr(ap, "bass_ap", None)
    if bap is not None:
        try:
            return (bap.tensor.name, bap.offset, str(bap.ap), str(ap.dtype))
        except AttributeError:
            return None
    try:
        return (ap.memref, ap.offset, str(ap.ap), str(ap.dtype))
    except AttributeError:
        return None


def _dedup_ldweights(ordered):
    """Drop InstLdweights that reload weights already resident in the PE
    array. Only drops clean instructions: no sync_info and sync-deps
    covered by the retained load."""
    n_drop = 0
    for bb, insts in ordered.items():
        out = []
        last_key = None
        last_deps = None
        for inst in insts:
            if isinstance(inst, mybir.InstLdweights):
                key = _ldw_key(inst)
                si = inst.sync_info
                clean = si is None or (not si.on_wait and not si.on_update)
                sdeps = set(inst.sync_dependency_names())
                nsdeps = set(inst.nosync_dependency_names())
                if (
                    key is not None
                    and key == last_key
                    and clean
                    and last_deps is not None
                    and sdeps <= last_deps
                    and not nsdeps
                ):
                    n_drop += 1
                    continue
                last_key = key
                last_deps = sdeps
            elif isinstance(inst, mybir.InstMatmult):
                pass  # does not clobber the weight array
            elif getattr(inst, "engine", None) == mybir.EngineType.PE:
                last_key = None
            out.append(inst)
        ordered[bb] = out
    return n_drop


def _build_bass(loops=1):
    import concourse.tile as tile_mod

    nc = bacc.Bacc()

    xp = nc.declare_dram_parameter("xp", [IMG_PER_CORE, 2, 128, L_PAD], FP16, isOutput=False)
    wt = nc.declare_dram_parameter("wt", [128, NW * 256], FP16, isOutput=False)
    bi = nc.declare_dram_parameter("bi", [2, 128, 1], F32, isOutput=False)
    y = nc.declare_dram_parameter("y", [IMG_PER_CORE, 2, 128, H * W], F32, isOutput=True)

    orig_legalize = tile_mod.tile_legalize

    def legalize_and_dedup(ordered, nc_arg):
        ordered = orig_legalize(ordered, nc_arg)
        _dedup_ldweights(ordered)
        return ordered

    tile_mod.tile_legalize = legalize_and_dedup
    try:
        _build_tile_program(nc, loops, xp, wt, bi, y)
    finally:
        tile_mod.tile_legalize = orig_legalize
    nc.compile()
    return nc


# phase-major x layout: padded plane rows stored as [rows 0,4,..,56 |
# 1,5,..,57 | 2,6,..,54 | 3,7,..,55], so the rows {4*hb+i : hb} that feed
# Winograd point views are flat contiguous 798-col slices.
_PHASE_BASE = (0, 15 * WP, 30 * WP, 44 * WP)  # block starts (15,15,14,14 rows)
_DOFF = (  # offset of d_i = rows 4*hb+i, hb=0..13
    _PHASE_BASE[0],
    _PHASE_BASE[1],
    _PHASE_BASE[2],
    _PHASE_BASE[3],
    _PHASE_BASE[0] + WP,
    _PHASE_BASE[1] + WP,
)


def _dview(xs, i):
    """x rows 4*hb + i (padded index) as a flat [128, 798] view."""
    return xs[:, LEAD + _DOFF[i] : LEAD + _DOFF[i] + UPL]


def _uview(us, u):
    return us[:, u * UL + U_LEAD : u * UL + U_LEAD + UPL]


def _fwd_transform(nc, xs, us, ts):
    """U_u = B^T d along H. DVE: u0,u3,u4,u5 (8 ops); GpSimd: u1,u2 (6 ops)."""
    A = mybir.AluOpType
    d = [_dview(xs, i) for i in range(6)]
    U = [_uview(us, u) for u in range(6)]
    T0 = ts[:, 0:UPL]
    T1 = ts[:, VL : VL + UPL]
    v, g = nc.vector, nc.gpsimd
    # u0 = -5*d2 + (4*d0 + d4)
    v.scalar_tensor_tensor(T0, d[0], 4.0, d[4], A.mult, A.add)
    v.scalar_tensor_tensor(U[0], d[2], -5.0, T0, A.mult, A.add)
    # u1 = -4*(d1+d2) + (d3+d4)
    g.tensor_add(U[1], d[1], d[2])
    g.tensor_add(T1, d[3], d[4])
    v.scalar_tensor_tensor(U[1], U[1], -4.0, T1, A.mult, A.add)
    # u2 = 4*(d1-d2) + (d4-d3)
    g.tensor_sub(U[2], d[1], d[2])
    g.tensor_sub(T1, d[4], d[3])
    v.scalar_tensor_tensor(U[2], U[2], 4.0, T1, A.mult, A.add)
    # u3 = 2*(d3-d1) + (d4-d2);  u4 = -2*(d3-d1) + (d4-d2)
    v.tensor_sub(U[3], d[3], d[1])
    v.tensor_sub(T0, d[4], d[2])
    v.scalar_tensor_tensor(U[4], U[3], -2.0, T0, A.mult, A.add)
    v.scalar_tensor_tensor(U[3], U[3], 2.0, T0, A.mult, A.add)
    # u5 = -5*d3 + (4*d1 + d5)
    v.scalar_tensor_tensor(T0, d[1], 4.0, d[5], A.mult, A.add)
    v.scalar_tensor_tensor(U[5], d[3], -5.0, T0, A.mult, A.add)


def _conv_kg(nc, pspool, it, vt, ot, usb0, usb1, wtile, bsb, y, n, kg):
    """GEMMs + inverse transform + bias/pack/DMA for one (img, kg)."""
    A = mybir.AluOpType
    usb = (usb0, usb1)

    def iv(j, ch):  # intermediate planes: a, b, m0, c, d
        return it[:, (j * 2 + ch) * 400 : (j * 2 + ch) * 400 + CHUNK]

    def vv(vi, ch):
        return vt[:, vi * VL + ch * CHUNK : vi * VL + (ch + 1) * CHUNK]

    for half in range(2):
        upts = (0, 1, 2) if half == 0 else (3, 4, 5)
        ps = {}
        for u in upts:
            for ch in range(2):
                ps[u, ch] = pspool.tile(
                    [128, CHUNK], F32, tag="ps", name=f"ps_{n}_{kg}_{u}_{ch}"
                )
        for u in upts:
            for si, (s, cgi) in enumerate(
                (s, c) for s in range(3) for c in range(2)
            ):
                wsl = wtile[
                    :,
                    ((u * 3 + s) * 2 + cgi) * 256
                    + kg * 128 : ((u * 3 + s) * 2 + cgi) * 256
                    + kg * 128
                    + 128,
                ]
                for ch in range(2):
                    base = u * UL + U_LEAD + ch * CHUNK + s - 1
                    nc.tensor.matmul(
                        ps[u, ch][:],
                        lhsT=wsl,
                        rhs=usb[cgi][:, base : base + CHUNK],
                        start=(si == 0),
                        stop=(si == 5),
                    )
        # NCC_IBVF027: tensor-tensor ops may read only one input from PSUM,
        # so stage M1/M3 into SBUF on the scalar engine first.
        if half == 0:
            for ch in range(2):
                m1s = iv(5, ch)
                nc.scalar.copy(m1s, ps[1, ch][:])
                nc.vector.tensor_add(iv(0, ch), m1s, ps[2, ch][:])
                nc.vector.tensor_sub(iv(1, ch), m1s, ps[2, ch][:])
                nc.scalar.copy(iv(2, ch), ps[0, ch][:])
        else:
            for ch in range(2):
                a_, b_, m0 = iv(0, ch), iv(1, ch), iv(2, ch)
                c_, d_ = iv(3, ch), iv(4, ch)
                m3s = iv(6, ch)
                nc.scalar.copy(m3s, ps[3, ch][:])
                nc.vector.tensor_add(c_, m3s, ps[4, ch][:])
                nc.vector.tensor_sub(d_, m3s, ps[4, ch][:])
                nc.gpsimd.tensor_add(vv(0, ch), m0, a_)
                nc.gpsimd.tensor_add(vv(0, ch), vv(0, ch), c_)
                nc.vector.scalar_tensor_tensor(vv(1, ch), d_, 2.0, b_, A.mult, A.add)
                nc.vector.scalar_tensor_tensor(vv(2, ch), c_, 4.0, a_, A.mult, A.add)
                nc.vector.scalar_tensor_tensor(vv(3, ch), d_, 8.0, b_, A.mult, A.add)
                nc.vector.tensor_add(vv(3, ch), vv(3, ch), ps[5, ch][:])
    for vi in range(4):
        in_v = vt[:, vi * VL : vi * VL + UPL].rearrange("p (hb w) -> p hb w", w=WP)[
            :, :, 1 : 1 + W
        ]
        out_v = ot.rearrange("p (hb f w) -> p hb f w", f=4, w=W)[:, :, vi, :]
        nc.scalar.activation(
            out_v, in_v, mybir.ActivationFunctionType.Identity, bias=bsb[kg]
        )
    nc.sync.dma_start(out=y[n, kg], in_=ot[:])


def _build_tile_program(nc, loops, xp, wt, bi, y):
    with TileContext(nc) as tc:
        with (
            tc.tile_pool(name="wpool", bufs=1) as wpool,
            tc.tile_pool(name="xpool", bufs=1) as xpool,
            tc.tile_pool(name="upool", bufs=1) as upool,
            tc.tile_pool(name="tpool", bufs=1) as tpool,
            tc.tile_pool(name="ipool", bufs=2) as ipool,
            tc.tile_pool(name="vpool", bufs=2) as vpool,
            tc.tile_pool(name="opool", bufs=3) as opool,
            tc.tile_pool(name="pspool", bufs=8, space="PSUM") as pspool,
        ):
            wtile = wpool.tile([128, NW * 256], FP16, tag="w")
            nc.sync.dma_start(out=wtile[:], in_=wt[:])
            bsb = []
            for cg in range(2):
                btile = wpool.tile([128, 1], F32, tag=f"b{cg}")
                nc.sync.dma_start(out=btile[:], in_=bi[cg])
                bsb.append(btile)

            xsb, usb, tsb = {}, {}, {}
            for slot in range(2):
                for cg in range(2):
                    xsb[slot, cg] = xpool.tile(
                        [128, LEAD + L_PAD + X_SLACK], FP16, tag=f"x{slot}{cg}", name=f"xsb{slot}{cg}"
                    )
                    nc.vector.memset(xsb[slot, cg][:], 0.0)
                    usb[slot, cg] = upool.tile([128, 6 * UL], FP16, tag=f"u{slot}{cg}", name=f"usb{slot}{cg}")
                    nc.vector.memset(usb[slot, cg][:], 0.0)
                    tsb[slot, cg] = tpool.tile([128, 2 * VL], FP16, tag=f"t{slot}{cg}", name=f"tsb{slot}{cg}")

            import contextlib

            loop_cm = (
                tc.For_i(0, loops, 1, hint_engines=(mybir.EngineType.PE,))
                if loops > 1
                else contextlib.nullcontext()
            )
            with loop_cm:
                for n in range(IMG_PER_CORE):
                    slot = n % 2
                    for cg in range(2):
                        nc.sync.dma_start(
                            out=xsb[slot, cg][:, LEAD : LEAD + L_PAD], in_=xp[n, cg]
                        )
                    for cg in range(2):
                        _fwd_transform(nc, xsb[slot, cg], usb[slot, cg], tsb[slot, cg])
                    for kg in range(2):
                        it = ipool.tile([128, 14 * 400], FP16, tag="i")
                        vt = vpool.tile([128, 4 * VL], FP16, tag="v")
                        ot = opool.tile([128, H * W], F32, tag="o")
                        _conv_kg(
                            nc, pspool, it, vt, ot,
                            usb[slot, 0], usb[slot, 1],
                            wtile, bsb, y, n, kg,
                        )


def _get_compiled(loops=1):
    key = (loops,)
    if key not in _compiled:
        _compiled[key] = _build_bass(loops)
    return _compiled[key]


def _prepare_inputs(x, weight, bias):
    x = np.asarray(x, dtype=np.float32)
    weight = np.asarray(weight, dtype=np.float32)
    bias = np.asarray(bias, dtype=np.float32)

    # padded pitch-57 fp16 activations, rows stored phase-major
    # (0,4,..,56 | 1,5,..,57 | 2,6,..,54 | 3,7,..,55)
    plane = np.zeros((N_IMG, C, HP, WP), dtype=np.float16)
    plane[:, :, 1 : 1 + H, 1 : 1 + W] = x.astype(np.float16)
    phase = np.concatenate(
        [plane[:, :, p::4, :] for p in range(4)], axis=2
    ).reshape(N_IMG, C, L_PLANE)
    xp = np.zeros((N_IMG, C, L_PAD), dtype=np.float16)
    xp[:, :, :L_PLANE] = phase

    # Winograd-transformed binarized weights:
    # wt[c', ((u*3+s)*2+cg)*256 + k] = sum_r G[u,r] sign(w)[k, cg*128+c', r, s]
    g = np.sign(weight)  # [K, C, 3, 3]
    Wt = np.einsum("ur,kcrs->ucsk", G_MAT, g.astype(np.float64))  # [6, C, 3, K]
    arr = Wt.reshape(6, 2, 128, 3, K).transpose(2, 0, 3, 1, 4)  # [128, 6, 3, 2, K]
    wt = np.ascontiguousarray(arr.reshape(128, NW * 256)).astype(np.float16)

    bi = bias.astype(np.float32).reshape(2, 128, 1)
    return xp, wt, bi


def kernel(x, weight, bias, _trace=False, _trace_kwargs=None):
    nc = _get_compiled()
    xp, wt, bi = _prepare_inputs(x, weight, bias)

    in_maps = []
    for i in range(N_CORES):
        xs = np.ascontiguousarray(
            xp[i * IMG_PER_CORE : (i + 1) * IMG_PER_CORE].reshape(
                IMG_PER_CORE, 2, 128, L_PAD
            )
        )
        in_maps.append({"xp": xs, "wt": wt, "bi": bi})

    res = run_bass_kernel_spmd(
        nc, in_maps, list(range(N_CORES)), trace=_trace, **(_trace_kwargs or {})
    )
    out = np.concatenate(
        [r["y"].reshape(IMG_PER_CORE, K, H, W) for r in res.results], axis=0
    )
    if _trace:
        return np.asarray(out, dtype=np.float32), res
    return np.asarray(out, dtype=np.float32)
